# revision 2
# baseline (speedup 1.0000x reference)
"""Weighted cross-entropy per-element loss on 8 Trainium2 NeuronCores.

loss[i] = -class_weights[target[i]] * log_softmax(logits, -1)[i, target[i]]

Sharding: pure data parallelism on the row (batch) axis — 1024 rows per
core, class_weights replicated.  Per core the kernel streams its logits
shard through SBUF once (f32, ~196 MiB => DMA-roofline bound), computes
exp + row-accumulate on the scalar engine in the same pass, reduces the
per-chunk partials on the vector engine, and fetches the two
data-dependent scalars per row (logits[r, t_r] and class_weights[t_r])
with gpsimd indirect-DMA gathers.

Gather-index guardrail: DVE integer adds go through the f32 datapath, so
any device-computed index must stay below 2^24.  Rows map block-major
(row = b*128 + p); the per-partition index is p*C + t (< 6.43M, exact)
and the per-block base b*128*C is supplied via the indirect DMA's
element_offset, which is applied in exact integer arithmetic.
"""

import numpy as np

import concourse.bacc as bacc
import concourse.tile as tile
from concourse import bass, mybir
from concourse.bass_utils import run_bass_kernel_spmd

N_CORES = 8
N_FULL = 8192
C_FULL = 50257
R = N_FULL // N_CORES  # rows per core
P = 128                # SBUF partitions
CHUNK = 8192           # logits columns per streamed tile (4 MiB DMAs)

_f32 = mybir.dt.float32
_i32 = mybir.dt.int32


def build_nc(r=R, c=C_FULL, chunk=CHUNK, stream_bufs=4):
    """Build + compile the per-core Bass program (same program on all cores).

    Row <-> (partition, block) mapping is row = b*P + p: streamed logits
    tiles are plain contiguous row slices; the small per-row tensors
    (targets, losses) use a transposed [(b p) -> p b] access pattern.
    """
    assert r % P == 0
    rb = r // P                     # row-blocks per core
    nchunk = -(-c // chunk)         # column chunks per row-block

    nc = bacc.Bacc("TRN2", target_bir_lowering=False)

    logits = nc.dram_tensor("logits", [r, c], _f32, kind="ExternalInput")
    tgt = nc.dram_tensor("tgt", [r], _i32, kind="ExternalInput")
    cw = nc.dram_tensor("cw", [c], _f32, kind="ExternalInput")
    loss = nc.dram_tensor("loss", [r], _f32, kind="ExternalOutput")

    # Compile-time constant p*C, replicated across block columns.
    rowoff_np = np.tile((np.arange(P)[:, None] * c).astype(np.int32), (1, rb))
    rowoff_dram = nc.inline_tensor(rowoff_np, name="rowoff")

    with tile.TileContext(nc) as tc:
        with (
            tc.tile_pool(name="stream", bufs=stream_bufs) as stream,
            tc.tile_pool(name="small", bufs=1) as small,
        ):
            partials = small.tile([P, rb, nchunk], _f32)
            sumexp = small.tile([P, rb], _f32)
            lse = small.tile([P, rb], _f32)
            tgt_t = small.tile([P, rb], _i32)
            rowoff_t = small.tile([P, rb], _i32)
            idx_t = small.tile([P, rb], _i32)
            picked = small.tile([P, rb], _f32)
            wt = small.tile([P, rb], _f32)
            out_t = small.tile([P, rb], _f32)

            nc.sync.dma_start(out=tgt_t[:], in_=tgt.rearrange("(b p) -> p b", p=P))
            nc.sync.dma_start(out=rowoff_t[:], in_=rowoff_dram[:])
            # idx[p, b] = tgt[b*P+p] + p*C  (< 2^24, exact on DVE's f32 path)
            nc.vector.tensor_tensor(
                out=idx_t[:], in0=tgt_t[:], in1=rowoff_t[:], op=mybir.AluOpType.add
            )

            for b in range(rb):
                # picked[p, b] = logits.flat[b*P*c + idx[p, b]]
                nc.gpsimd.indirect_dma_start(
                    out=picked[:, b : b + 1],
                    out_offset=None,
                    in_=logits[:],
                    in_offset=bass.IndirectOffsetOnAxis(
                        ap=idx_t[:, b : b + 1], axis=1
                    ),
                    element_offset=b * P * c,
                )
                # wt[p, b] = cw[tgt[b*P+p]]
                nc.gpsimd.indirect_dma_start(
                    out=wt[:, b : b + 1],
                    out_offset=None,
                    in_=cw[:, None],
                    in_offset=bass.IndirectOffsetOnAxis(
                        ap=tgt_t[:, b : b + 1], axis=0
                    ),
                )

            for b in range(rb):
                for j in range(nchunk):
                    j0 = j * chunk
                    w = min(chunk, c - j0)
                    t = stream.tile([P, chunk], _f32, tag="stream")
                    nc.sync.dma_start(
                        out=t[:, :w], in_=logits[b * P : (b + 1) * P, j0 : j0 + w]
                    )
                    nc.scalar.activation(
                        out=t[:, :w],
                        in_=t[:, :w],
                        func=mybir.ActivationFunctionType.Exp,
                        accum_out=partials[:, b, j : j + 1],
                    )

            nc.vector.tensor_reduce(
                out=sumexp[:],
                in_=partials[:],
                axis=mybir.AxisListType.X,
                op=mybir.AluOpType.add,
            )
            nc.scalar.activation(
                out=lse[:], in_=sumexp[:], func=mybir.ActivationFunctionType.Ln
            )
            # loss = wt * (lse - picked)
            nc.vector.tensor_tensor(
                out=out_t[:], in0=lse[:], in1=picked[:], op=mybir.AluOpType.subtract
            )
            nc.vector.tensor_tensor(
                out=out_t[:], in0=out_t[:], in1=wt[:], op=mybir.AluOpType.mult
            )
            nc.sync.dma_start(
                out=loss.rearrange("(b p) -> p b", p=P), in_=out_t[:]
            )

    nc.compile()
    return nc


_compiled_nc = None
last_results = None  # BassKernelResults of the most recent run (for profiling)


def kernel(logits, target, class_weights):
    global _compiled_nc, last_results

    logits = np.ascontiguousarray(np.asarray(logits), dtype=np.float32)
    tgt = np.ascontiguousarray(np.asarray(target).astype(np.int32))
    cw = np.ascontiguousarray(np.asarray(class_weights), dtype=np.float32)
    assert logits.shape == (N_FULL, C_FULL), logits.shape

    if _compiled_nc is None:
        _compiled_nc = build_nc()

    in_maps = [
        {
            "logits": logits[k * R : (k + 1) * R],
            "tgt": tgt[k * R : (k + 1) * R],
            "cw": cw,
        }
        for k in range(N_CORES)
    ]
    last_results = run_bass_kernel_spmd(
        _compiled_nc, in_maps, core_ids=list(range(N_CORES))
    )
    return np.concatenate(
        [last_results.results[k]["loss"] for k in range(N_CORES)]
    ).astype(np.float32)


# revision 5
# speedup vs baseline: 1.0859x; 1.0859x over previous
"""Weighted cross-entropy per-element loss on 8 Trainium2 NeuronCores.

loss[i] = -class_weights[target[i]] * log_softmax(logits, -1)[i, target[i]]

Sharding: pure data parallelism on the row (batch) axis — 1024 rows per
core, class_weights replicated.  Per core the kernel streams its logits
shard through SBUF once (f32, ~196 MiB => DMA-roofline bound), computes
exp + row-accumulate on the scalar engine in the same pass, reduces the
per-chunk partials on the vector engine, and fetches the two
data-dependent scalars per row (logits[r, t_r] and class_weights[t_r])
with gpsimd indirect-DMA gathers.

Gather-index guardrail: DVE integer adds go through the f32 datapath, so
any device-computed index must stay below 2^24.  Rows map block-major
(row = b*128 + p); the per-partition index is p*C + t (< 6.43M, exact)
and the per-block base b*128*C is supplied via the indirect DMA's
element_offset, which is applied in exact integer arithmetic.
"""

import numpy as np

import concourse.bacc as bacc
import concourse.tile as tile
from concourse import bass, mybir
from concourse.bass_utils import run_bass_kernel_spmd

N_CORES = 8
N_FULL = 8192
C_FULL = 50257
R = N_FULL // N_CORES  # rows per core
P = 128                # SBUF partitions
CHUNK = 8192           # logits columns per streamed tile (4 MiB DMAs)

_f32 = mybir.dt.float32
_i32 = mybir.dt.int32


def _chunk_widths(c, chunk, taper):
    """Column-chunk widths for one row-block. With taper=True the last
    ~2*chunk columns are split into halving pieces so the final exp (which
    can only start after the last DMA lands) is short — that exp is on the
    kernel's critical path."""
    widths = []
    left = c
    limit = 2 * chunk if taper else chunk
    while left > limit:
        widths.append(chunk)
        left -= chunk
    if not taper:
        if left:
            widths.append(left)
        return widths
    w = chunk
    while left > 0:
        w = max(w // 2, chunk // 8)
        take = min(w, left)
        widths.append(take)
        left -= take
    return widths


def build_nc(r=R, c=C_FULL, chunk=CHUNK, stream_bufs=5):
    """Build + compile the per-core Bass program (same program on all cores).

    Row <-> (partition, block) mapping is row = b*P + p: streamed logits
    tiles are plain contiguous row slices; the small per-row tensors
    (targets, losses) use a transposed [(b p) -> p b] access pattern.
    """
    assert r % P == 0
    rb = r // P                     # row-blocks per core
    nchunk = max(len(_chunk_widths(c, chunk, True)), len(_chunk_widths(c, chunk, False)))

    nc = bacc.Bacc("TRN2", target_bir_lowering=False)

    logits = nc.dram_tensor("logits", [r, c], _f32, kind="ExternalInput")
    tgt = nc.dram_tensor("tgt", [r], _i32, kind="ExternalInput")
    cw = nc.dram_tensor("cw", [c], _f32, kind="ExternalInput")
    loss = nc.dram_tensor("loss", [r], _f32, kind="ExternalOutput")

    # Compile-time constant p*C, replicated across block columns.
    rowoff_np = np.tile((np.arange(P)[:, None] * c).astype(np.int32), (1, rb))
    rowoff_dram = nc.inline_tensor(rowoff_np, name="rowoff")

    with tile.TileContext(nc) as tc:
        with (
            tc.tile_pool(name="stream", bufs=stream_bufs) as stream,
            tc.tile_pool(name="small", bufs=1) as small,
        ):
            partials = small.tile([P, rb, nchunk], _f32)
            sumexp = small.tile([P, rb], _f32)
            lse = small.tile([P, rb], _f32)
            tgt_t = small.tile([P, rb], _i32)
            rowoff_t = small.tile([P, rb], _i32)
            idx_t = small.tile([P, rb], _i32)
            picked = small.tile([P, rb], _f32)
            wt = small.tile([P, rb], _f32)
            out_t = small.tile([P, rb], _f32)

            # Stream first so the big DMAs head the HWDGE ring; the small
            # setup loads go through gpsimd's software DGE instead.
            for b in range(rb):
                widths = _chunk_widths(c, chunk, taper=(b == rb - 1))
                j0 = 0
                for j, w in enumerate(widths):
                    t = stream.tile([P, chunk], _f32, tag="stream")
                    nc.sync.dma_start(
                        out=t[:, :w], in_=logits[b * P : (b + 1) * P, j0 : j0 + w]
                    )
                    nc.scalar.activation(
                        out=t[:, :w],
                        in_=t[:, :w],
                        func=mybir.ActivationFunctionType.Exp,
                        accum_out=partials[:, b, j : j + 1],
                    )
                    j0 += w
                if len(widths) < nchunk:
                    nc.vector.memset(partials[:, b, len(widths) : nchunk], 0.0)

            nc.gpsimd.dma_start(out=tgt_t[:], in_=tgt.rearrange("(b p) -> p b", p=P))
            nc.gpsimd.dma_start(out=rowoff_t[:], in_=rowoff_dram[:])
            # idx[p, b] = tgt[b*P+p] + p*C  (< 2^24, exact on DVE's f32 path)
            nc.vector.tensor_tensor(
                out=idx_t[:], in0=tgt_t[:], in1=rowoff_t[:], op=mybir.AluOpType.add
            )

            for b in range(rb):
                # picked[p, b] = logits.flat[b*P*c + idx[p, b]]
                nc.gpsimd.indirect_dma_start(
                    out=picked[:, b : b + 1],
                    out_offset=None,
                    in_=logits[:],
                    in_offset=bass.IndirectOffsetOnAxis(
                        ap=idx_t[:, b : b + 1], axis=1
                    ),
                    element_offset=b * P * c,
                )
                # wt[p, b] = cw[tgt[b*P+p]]
                nc.gpsimd.indirect_dma_start(
                    out=wt[:, b : b + 1],
                    out_offset=None,
                    in_=cw[:, None],
                    in_offset=bass.IndirectOffsetOnAxis(
                        ap=tgt_t[:, b : b + 1], axis=0
                    ),
                )

            nc.vector.tensor_reduce(
                out=sumexp[:],
                in_=partials[:],
                axis=mybir.AxisListType.X,
                op=mybir.AluOpType.add,
            )
            nc.scalar.activation(
                out=lse[:], in_=sumexp[:], func=mybir.ActivationFunctionType.Ln
            )
            # loss = wt * (lse - picked)
            nc.vector.tensor_tensor(
                out=out_t[:], in0=lse[:], in1=picked[:], op=mybir.AluOpType.subtract
            )
            nc.vector.tensor_tensor(
                out=out_t[:], in0=out_t[:], in1=wt[:], op=mybir.AluOpType.mult
            )
            nc.sync.dma_start(
                out=loss.rearrange("(b p) -> p b", p=P), in_=out_t[:]
            )

    nc.compile()
    return nc


_compiled_nc = None
last_results = None  # BassKernelResults of the most recent run (for profiling)


def kernel(logits, target, class_weights):
    global _compiled_nc, last_results

    logits = np.ascontiguousarray(np.asarray(logits), dtype=np.float32)
    tgt = np.ascontiguousarray(np.asarray(target).astype(np.int32))
    cw = np.ascontiguousarray(np.asarray(class_weights), dtype=np.float32)
    assert logits.shape == (N_FULL, C_FULL), logits.shape

    if _compiled_nc is None:
        _compiled_nc = build_nc()

    in_maps = [
        {
            "logits": logits[k * R : (k + 1) * R],
            "tgt": tgt[k * R : (k + 1) * R],
            "cw": cw,
        }
        for k in range(N_CORES)
    ]
    last_results = run_bass_kernel_spmd(
        _compiled_nc, in_maps, core_ids=list(range(N_CORES))
    )
    return np.concatenate(
        [last_results.results[k]["loss"] for k in range(N_CORES)]
    ).astype(np.float32)


# revision 7
# speedup vs baseline: 1.0944x; 1.0078x over previous
"""Weighted cross-entropy per-element loss on 8 Trainium2 NeuronCores.

loss[i] = -class_weights[target[i]] * log_softmax(logits, -1)[i, target[i]]

Sharding: pure data parallelism on the row (batch) axis — 1024 rows per
core, class_weights replicated.  Per core the kernel streams its logits
shard through SBUF once (f32, ~196 MiB => DMA-roofline bound), computes
exp + row-accumulate on the scalar engine in the same pass, reduces the
per-chunk partials on the vector engine, and fetches the two
data-dependent scalars per row (logits[r, t_r] and class_weights[t_r])
with gpsimd indirect-DMA gathers.

Gather-index guardrail: DVE integer adds go through the f32 datapath, so
any device-computed index must stay below 2^24.  Rows map block-major
(row = b*128 + p); the per-partition index is p*C + t (< 6.43M, exact)
and the per-block base b*128*C is supplied via the indirect DMA's
element_offset, which is applied in exact integer arithmetic.
"""

import numpy as np

import concourse.bacc as bacc
import concourse.tile as tile
from concourse import bass, mybir
from concourse.bass_utils import run_bass_kernel_spmd

N_CORES = 8
N_FULL = 8192
C_FULL = 50257
R = N_FULL // N_CORES  # rows per core
P = 128                # SBUF partitions
CHUNK = 8192           # logits columns per streamed tile (4 MiB DMAs)

_f32 = mybir.dt.float32
_i32 = mybir.dt.int32


def _chunk_widths(c, chunk, taper):
    """Column-chunk widths for one row-block. With taper=True the last
    ~2*chunk columns are split into halving pieces so the final exp (which
    can only start after the last DMA lands) is short — that exp is on the
    kernel's critical path."""
    widths = []
    left = c
    limit = 2 * chunk if taper else chunk
    while left > limit:
        widths.append(chunk)
        left -= chunk
    if not taper:
        if left:
            widths.append(left)
        return widths
    w = chunk
    while left > 0:
        w = max(w // 2, chunk // 16)
        take = min(w, left)
        widths.append(take)
        left -= take
    return widths


def build_nc(r=R, c=C_FULL, chunk=CHUNK, stream_bufs=5):
    """Build + compile the per-core Bass program (same program on all cores).

    Row <-> (partition, block) mapping is row = b*P + p: streamed logits
    tiles are plain contiguous row slices; the small per-row tensors
    (targets, losses) use a transposed [(b p) -> p b] access pattern.
    """
    assert r % P == 0
    rb = r // P                     # row-blocks per core
    nchunk = max(len(_chunk_widths(c, chunk, True)), len(_chunk_widths(c, chunk, False)))

    nc = bacc.Bacc("TRN2", target_bir_lowering=False)

    logits = nc.dram_tensor("logits", [r, c], _f32, kind="ExternalInput")
    tgt = nc.dram_tensor("tgt", [r], _i32, kind="ExternalInput")
    cw = nc.dram_tensor("cw", [c], _f32, kind="ExternalInput")
    loss = nc.dram_tensor("loss", [r], _f32, kind="ExternalOutput")

    # Compile-time constant p*C, replicated across block columns.
    rowoff_np = np.tile((np.arange(P)[:, None] * c).astype(np.int32), (1, rb))
    rowoff_dram = nc.inline_tensor(rowoff_np, name="rowoff")

    with tile.TileContext(nc) as tc:
        with (
            tc.tile_pool(name="stream", bufs=stream_bufs) as stream,
            tc.tile_pool(name="small", bufs=1) as small,
        ):
            partials = small.tile([P, rb, nchunk], _f32)
            sumexp = small.tile([P, rb], _f32)
            lse = small.tile([P, rb], _f32)
            tgt_t = small.tile([P, rb], _i32)
            rowoff_t = small.tile([P, rb], _i32)
            idx_t = small.tile([P, rb], _i32)
            picked = small.tile([P, rb], _f32)
            wt = small.tile([P, rb], _f32)
            out_t = small.tile([P, rb], _f32)

            # Warm the ACT table with Ln first: natural_log_exp_and_others
            # holds both Ln and Exp, so the final Ln needs no table reload
            # (a table switch on the critical path costs ~2.7us).
            ln_warm = small.tile([P, 1], _f32)
            nc.gpsimd.memset(ln_warm[:], 1.0)
            nc.scalar.activation(
                out=ln_warm[:], in_=ln_warm[:], func=mybir.ActivationFunctionType.Ln
            )

            # Stream first so the big DMAs head the HWDGE ring; the small
            # setup loads go through gpsimd's software DGE instead.
            for b in range(rb):
                widths = _chunk_widths(c, chunk, taper=(b == rb - 1))
                j0 = 0
                for j, w in enumerate(widths):
                    t = stream.tile([P, chunk], _f32, tag="stream")
                    nc.sync.dma_start(
                        out=t[:, :w], in_=logits[b * P : (b + 1) * P, j0 : j0 + w]
                    )
                    nc.scalar.activation(
                        out=t[:, :w],
                        in_=t[:, :w],
                        func=mybir.ActivationFunctionType.Exp,
                        accum_out=partials[:, b, j : j + 1],
                    )
                    j0 += w
                if len(widths) < nchunk:
                    nc.vector.memset(partials[:, b, len(widths) : nchunk], 0.0)

            nc.gpsimd.dma_start(out=tgt_t[:], in_=tgt.rearrange("(b p) -> p b", p=P))
            nc.gpsimd.dma_start(out=rowoff_t[:], in_=rowoff_dram[:])
            # idx[p, b] = tgt[b*P+p] + p*C  (< 2^24, exact on DVE's f32 path)
            nc.vector.tensor_tensor(
                out=idx_t[:], in0=tgt_t[:], in1=rowoff_t[:], op=mybir.AluOpType.add
            )

            for b in range(rb):
                # picked[p, b] = logits.flat[b*P*c + idx[p, b]]
                nc.gpsimd.indirect_dma_start(
                    out=picked[:, b : b + 1],
                    out_offset=None,
                    in_=logits[:],
                    in_offset=bass.IndirectOffsetOnAxis(
                        ap=idx_t[:, b : b + 1], axis=1
                    ),
                    element_offset=b * P * c,
                )
                # wt[p, b] = cw[tgt[b*P+p]]
                nc.gpsimd.indirect_dma_start(
                    out=wt[:, b : b + 1],
                    out_offset=None,
                    in_=cw[:, None],
                    in_offset=bass.IndirectOffsetOnAxis(
                        ap=tgt_t[:, b : b + 1], axis=0
                    ),
                )

            nc.vector.tensor_reduce(
                out=sumexp[:],
                in_=partials[:],
                axis=mybir.AxisListType.X,
                op=mybir.AluOpType.add,
            )
            nc.scalar.activation(
                out=lse[:], in_=sumexp[:], func=mybir.ActivationFunctionType.Ln
            )
            # loss = wt * (lse - picked)
            nc.vector.tensor_tensor(
                out=out_t[:], in0=lse[:], in1=picked[:], op=mybir.AluOpType.subtract
            )
            nc.vector.tensor_tensor(
                out=out_t[:], in0=out_t[:], in1=wt[:], op=mybir.AluOpType.mult
            )
            nc.sync.dma_start(
                out=loss.rearrange("(b p) -> p b", p=P), in_=out_t[:]
            )

    nc.compile()
    return nc


_compiled_nc = None
last_results = None  # BassKernelResults of the most recent run (for profiling)


def kernel(logits, target, class_weights):
    global _compiled_nc, last_results

    logits = np.ascontiguousarray(np.asarray(logits), dtype=np.float32)
    tgt = np.ascontiguousarray(np.asarray(target).astype(np.int32))
    cw = np.ascontiguousarray(np.asarray(class_weights), dtype=np.float32)
    assert logits.shape == (N_FULL, C_FULL), logits.shape

    if _compiled_nc is None:
        _compiled_nc = build_nc()

    in_maps = [
        {
            "logits": logits[k * R : (k + 1) * R],
            "tgt": tgt[k * R : (k + 1) * R],
            "cw": cw,
        }
        for k in range(N_CORES)
    ]
    last_results = run_bass_kernel_spmd(
        _compiled_nc, in_maps, core_ids=list(range(N_CORES))
    )
    return np.concatenate(
        [last_results.results[k]["loss"] for k in range(N_CORES)]
    ).astype(np.float32)
